# revision 12
# baseline (speedup 1.0000x reference)
"""Trainium2 Bass kernel for nn_Attention_82867099009484 (sparse_attention).

Reference computation (B=16, S=4096, H=1024, Q=1024, K=2048):
    q      = query @ Wq.T                      [B,1,H]
    scores = tanh(q + proj_key) . w_energy     [B,S]
    scores = where(mask==0, -inf, scores)
    alphas = softmax(scores)                   [B,1,S]
    ctx    = alphas @ value                    [B,1,K]
    return (ctx, alphas)

Strategy: data-parallel over batch across 8 NeuronCores (2 batches/core).
Per core, stream proj_key/value row-chunks of 128 seq positions with S on
SBUF partitions:
  - DVE adds broadcast q, ScalarE computes tanh, DVE tensor_tensor_reduce
    fuses the w_energy multiply + h-reduction into one pass -> scores.
  - Softmax without max-subtraction (scores bounded by sum|w| ~ 16, exp
    cannot overflow f32); the mask enters as an additive bias (0 / -1e30)
    folded into the Exp activation's per-partition bias operand.
  - TensorE accumulates ctx = sum_s exp_s * value_s into PSUM chunk by
    chunk (unnormalized); everything is scaled by 1/total at the end.
Host side only reshapes/transposes small tensors (Wq 4MB, query 64KB,
mask 256KB) and assembles the sharded outputs.

Sparse mode: mask sparsity (~50%) means masked rows contribute nothing.
Host computes per-batch lists of unmasked row indices; the device gathers
only those rows of proj_key/value via dma_gather, halving HBM traffic.
"""
import sys, os, time

for _p in ("/opt/trn_rl_repo", "/root/.axon_site/_ro/trn_rl_repo"):
    if os.path.isdir(_p) and _p not in sys.path:
        sys.path.append(_p)

import numpy as np

B, S, H, Q, KV = 16, 4096, 1024, 1024, 2048
N_CORES = 8
B_LOC = B // N_CORES          # 2 batches per core
N_CHUNK = S // 128            # 32 chunks of 128 seq rows (dense)
NEG = -1.0e30

_RUNNER_CACHE = {}


def _build(cfg):
    """Build + bacc-compile the Bass graph. cfg keys: k_iters, f32r, c_pad."""
    import concourse.bacc as bacc
    import concourse.tile as tile
    import concourse.bass as bass
    from concourse import mybir

    F32 = mybir.dt.float32
    F32R = mybir.dt.float32r
    I16 = mybir.dt.int16
    AF = mybir.ActivationFunctionType
    ALU = mybir.AluOpType
    AX = mybir.AxisListType

    k_iters = cfg["k_iters"]
    mm_dt = F32R if cfg["f32r"] else F32
    c_pad = cfg["c_pad"]          # 0 => dense; else padded gather count
    sparse = c_pad > 0
    n_chunk = (c_pad // 128) if sparse else N_CHUNK
    G_ROWS = 512                  # rows per dma_gather instruction
    assert not sparse or c_pad % G_ROWS == 0

    nc = bacc.Bacc("TRN2", target_bir_lowering=False, num_devices=N_CORES)

    pk = nc.dram_tensor("proj_key", [B_LOC, S, H], F32, kind="ExternalInput").ap()
    val = nc.dram_tensor("value", [B_LOC, S, KV], mm_dt, kind="ExternalInput").ap()
    wqt = nc.dram_tensor("wq_t", [Q, H], mm_dt, kind="ExternalInput").ap()
    qpt = nc.dram_tensor("query_pt", [128, (Q // 128) * B_LOC], mm_dt,
                         kind="ExternalInput").ap()
    wen = nc.dram_tensor("w_energy", [H], F32, kind="ExternalInput").ap()
    mb = nc.dram_tensor("mask_bias", [B_LOC, 128, n_chunk], F32,
                        kind="ExternalInput").ap()
    if sparse:
        idx = nc.dram_tensor("gather_idx", [B_LOC, 128, c_pad // 16], I16,
                             kind="ExternalInput").ap()
    ctx_o = nc.dram_tensor("ctx", [B_LOC, KV], F32, kind="ExternalOutput").ap()
    al_o = nc.dram_tensor("alphas_c", [B_LOC, 128, n_chunk], F32,
                          kind="ExternalOutput").ap()

    with tile.TileContext(nc) as tc:
        from contextlib import ExitStack
        with ExitStack() as ctx:
            const = ctx.enter_context(tc.tile_pool(name="const", bufs=1))
            dma_bufs = 2 if sparse else 3
            pkp = ctx.enter_context(tc.tile_pool(name="pkp", bufs=dma_bufs))
            vp = ctx.enter_context(tc.tile_pool(name="vp", bufs=dma_bufs))
            sump = ctx.enter_context(tc.tile_pool(name="sump", bufs=3))
            tanhp = ctx.enter_context(tc.tile_pool(name="tanhp", bufs=3))
            smalls = ctx.enter_context(tc.tile_pool(name="smalls", bufs=2))
            outp = ctx.enter_context(tc.tile_pool(name="outp", bufs=2))
            psq = ctx.enter_context(tc.tile_pool(name="psq", bufs=1, space="PSUM"))
            pstot = ctx.enter_context(tc.tile_pool(name="pstot", bufs=1, space="PSUM"))
            psctx = ctx.enter_context(tc.tile_pool(name="psctx", bufs=1, space="PSUM"))

            # ---- constants (loaded once, reused across iterations) ----
            wqt_sb = const.tile([128, Q // 128, H], mm_dt)   # (p, t, h) = wq_t[t*128+p, h]
            nc.sync.dma_start(wqt_sb[:], wqt.rearrange("(t p) h -> p t h", p=128))
            qpt_sb = const.tile([128, (Q // 128) * B_LOC], mm_dt)
            nc.sync.dma_start(qpt_sb[:], qpt[:])
            wb_sb = const.tile([128, H], F32)                # w_energy bcast over partitions
            nc.gpsimd.dma_start(
                out=wb_sb[:],
                in_=bass.AP(tensor=wen.tensor, offset=wen.offset,
                            ap=[[0, 128]] + list(wen.ap)))
            ones_sb = const.tile([128, 128], F32)
            nc.vector.memset(ones_sb[:], 1.0)
            ones1_sb = const.tile([1, 128], F32)
            nc.vector.memset(ones1_sb[:], 1.0)
            mb_sb = const.tile([128, B_LOC, n_chunk], F32)
            for b in range(B_LOC):
                nc.sync.dma_start(mb_sb[:, b, :], mb[b])
            if sparse:
                idx_sb = const.tile([128, B_LOC, c_pad // 16], I16)
                for b in range(B_LOC):
                    nc.sync.dma_start(idx_sb[:, b, :], idx[b])

            for it in range(k_iters):
                for b in range(B_LOC):
                    # ---- q = query[b] @ Wq.T, as [1, H] in PSUM ----
                    ps_q = psq.tile([1, H], F32, tag="psq_shared")
                    for hc in range(H // 512):
                        for t in range(Q // 128):
                            nc.tensor.matmul(
                                ps_q[0:1, hc * 512:(hc + 1) * 512],
                                lhsT=qpt_sb[:, (t * B_LOC + b):(t * B_LOC + b + 1)],
                                rhs=wqt_sb[:, t, hc * 512:(hc + 1) * 512],
                                start=(t == 0), stop=(t == Q // 128 - 1))
                    q_sb = smalls.tile([1, H], F32)
                    nc.vector.tensor_copy(q_sb[:], ps_q[:])
                    # broadcast across partitions: ones[1,128].T @ q_sb[1,H]
                    ps_qb = psq.tile([128, H], F32, tag="psq_shared")
                    for kk in range(H // 512):
                        nc.tensor.matmul(ps_qb[:, kk * 512:(kk + 1) * 512],
                                         lhsT=ones1_sb[:],
                                         rhs=q_sb[:, kk * 512:(kk + 1) * 512],
                                         start=True, stop=True)
                    qb_sb = smalls.tile([128, H], F32)
                    nc.vector.tensor_copy(qb_sb[:], ps_qb[:])

                    exp_sb = smalls.tile([128, n_chunk], mm_dt)
                    scores_sb = smalls.tile([128, n_chunk], F32)
                    ps_ctx = psctx.tile([1, KV], F32)

                    if sparse:
                        # gather unmasked rows, G_ROWS per dma_gather
                        pk_g = []
                        v_g = []
                        for g in range(c_pad // G_ROWS):
                            pkt = pkp.tile([128, G_ROWS // 128, H], F32)
                            nc.gpsimd.dma_gather(
                                out_ap=pkt[:], in_ap=pk[b],
                                idxs_ap=idx_sb[:, b, g * (G_ROWS // 16):(g + 1) * (G_ROWS // 16)],
                                num_idxs=G_ROWS, num_idxs_reg=G_ROWS,
                                elem_size=H, elem_step=H)
                            vt = vp.tile([128, G_ROWS // 128, KV], mm_dt)
                            nc.gpsimd.dma_gather(
                                out_ap=vt[:], in_ap=val[b],
                                idxs_ap=idx_sb[:, b, g * (G_ROWS // 16):(g + 1) * (G_ROWS // 16)],
                                num_idxs=G_ROWS, num_idxs_reg=G_ROWS,
                                elem_size=KV, elem_step=KV)
                            pk_g.append(pkt)
                            v_g.append(vt)

                    for c in range(n_chunk):
                        if sparse:
                            pk_t = pk_g[c // (G_ROWS // 128)][:, c % (G_ROWS // 128), :]
                            v_t = v_g[c // (G_ROWS // 128)][:, c % (G_ROWS // 128), :]
                        else:
                            pk_t_full = pkp.tile([128, H], F32)
                            nc.sync.dma_start(pk_t_full[:], pk[b, c * 128:(c + 1) * 128, :])
                            v_t_full = vp.tile([128, KV], mm_dt)
                            nc.scalar.dma_start(v_t_full[:], val[b, c * 128:(c + 1) * 128, :])
                            pk_t = pk_t_full[:]
                            v_t = v_t_full[:]

                        sum_t = sump.tile([128, H], F32)
                        nc.vector.tensor_add(sum_t[:], pk_t, qb_sb[:])
                        tanh_t = tanhp.tile([128, H], F32)
                        nc.scalar.activation(tanh_t[:], sum_t[:], AF.Tanh)
                        # sum_t reused as dummy elementwise output;
                        # accum_out = sum_h(tanh * w_energy) in one DVE pass
                        nc.vector.scalar_tensor_tensor(
                            out=sum_t[:], in0=tanh_t[:], scalar=1.0,
                            in1=wb_sb[:], op0=ALU.mult, op1=ALU.mult,
                            accum_out=scores_sb[:, c:c + 1])
                        # exp(scores + mask_bias); masked/pad rows -> exp(-1e30) = 0
                        nc.scalar.activation(exp_sb[:, c:c + 1], scores_sb[:, c:c + 1],
                                             AF.Exp, bias=mb_sb[:, b, c:c + 1])
                        for k in range(KV // 512):
                            nc.tensor.matmul(
                                ps_ctx[0:1, k * 512:(k + 1) * 512],
                                lhsT=exp_sb[:, c:c + 1],
                                rhs=v_t[:, k * 512:(k + 1) * 512],
                                start=(c == 0), stop=(c == n_chunk - 1))

                    # ---- softmax denominator + normalization ----
                    sums = smalls.tile([128, 1], F32)
                    exp_f32 = exp_sb[:].bitcast(F32)
                    nc.vector.reduce_sum(sums[:], exp_f32, axis=AX.X)
                    ps_tot = pstot.tile([128, 1], F32)
                    nc.tensor.matmul(ps_tot[:], lhsT=ones_sb[:], rhs=sums[:],
                                     start=True, stop=True)
                    inv_sb = smalls.tile([128, 1], F32)
                    nc.vector.reciprocal(inv_sb[:], ps_tot[:])
                    al_sb = outp.tile([128, n_chunk], F32)
                    nc.vector.tensor_scalar_mul(al_sb[:], exp_f32, inv_sb[:, 0:1])
                    nc.sync.dma_start(al_o[b], al_sb[:])
                    for k in range(KV // 512):
                        ctx_sb = outp.tile([1, 512], F32)
                        nc.vector.tensor_scalar_mul(
                            ctx_sb[:], ps_ctx[0:1, k * 512:(k + 1) * 512],
                            inv_sb[0:1, 0:1])
                        nc.sync.dma_start(ctx_o[b:b + 1, k * 512:(k + 1) * 512],
                                          ctx_sb[:])

    nc.compile()
    return nc


def _make_runner(cfg):
    """Compile once; return f(in_maps) -> (fn, dargs-builder) reusable callable."""
    import jax
    from jax.sharding import Mesh, PartitionSpec
    from jax.experimental.shard_map import shard_map
    from concourse import mybir
    from concourse.bass2jax import (_bass_exec_p, partition_id_tensor,
                                    install_neuronx_cc_hook)

    nc = _build(cfg)
    install_neuronx_cc_hook()
    partition_name = nc.partition_id_tensor.name if nc.partition_id_tensor else None
    in_names, out_names, out_avals, zero_outs = [], [], [], []
    for alloc in nc.m.functions[0].allocations:
        if not isinstance(alloc, mybir.MemoryLocationSet):
            continue
        name = alloc.memorylocations[0].name
        if alloc.kind == "ExternalInput":
            if name != partition_name:
                in_names.append(name)
        elif alloc.kind == "ExternalOutput":
            out_names.append(name)
            out_avals.append(jax.core.ShapedArray(
                tuple(alloc.tensor_shape), mybir.dt.np(alloc.dtype)))
            zero_outs.append(np.zeros(tuple(alloc.tensor_shape),
                                      mybir.dt.np(alloc.dtype)))
    n_params = len(in_names)
    all_in = list(in_names) + list(out_names) + (
        [partition_name] if partition_name else [])

    def _body(*args):
        ops = list(args)
        if partition_name:
            ops.append(partition_id_tensor())
        return tuple(_bass_exec_p.bind(
            *ops, out_avals=tuple(out_avals), in_names=tuple(all_in),
            out_names=tuple(out_names), lowering_input_output_aliases=(),
            sim_require_finite=True, sim_require_nnan=True, nc=nc))

    import jax
    devices = jax.devices()[:N_CORES]
    mesh = Mesh(np.asarray(devices), ("core",))
    nio = n_params + len(out_names)
    fn = jax.jit(shard_map(_body, mesh=mesh,
                           in_specs=(PartitionSpec("core"),) * nio,
                           out_specs=(PartitionSpec("core"),) * len(out_names),
                           check_rep=False))

    def run(in_maps, reps=1):
        from jax.sharding import NamedSharding
        sh = NamedSharding(mesh, PartitionSpec("core"))

        def to_global(per_core_arrs):
            """Build a sharded global array from per-device shards without
            going through jax's _multi_slice path."""
            shards = [jax.device_put(a, d)
                      for a, d in zip(per_core_arrs, devices)]
            shape = (sum(a.shape[0] for a in per_core_arrs),) + per_core_arrs[0].shape[1:]
            return jax.make_array_from_single_device_arrays(shape, sh, shards)

        dargs = []
        for i, nm in enumerate(in_names):
            dargs.append(to_global([np.asarray(m[nm]) for m in in_maps]))
        for z in zero_outs:
            dargs.append(to_global([z] * N_CORES))
        jax.block_until_ready(dargs)
        outs = fn(*dargs)
        jax.block_until_ready(outs)          # warm call
        times = []
        for _ in range(reps):
            t0 = time.perf_counter()
            outs = fn(*dargs)
            jax.block_until_ready(outs)
            times.append(time.perf_counter() - t0)
        res = []
        for c in range(N_CORES):
            m = {}
            for i, nm in enumerate(out_names):
                arr = np.asarray(outs[i])
                per = arr.shape[0] // N_CORES
                m[nm] = arr[c * per:(c + 1) * per]
            res.append(m)
        return res, times

    return run


def get_runner(k_iters=1, f32r=True, c_pad=0):
    key = (k_iters, f32r, c_pad)
    if key not in _RUNNER_CACHE:
        _RUNNER_CACHE[key] = _make_runner(
            {"k_iters": k_iters, "f32r": f32r, "c_pad": c_pad})
    return _RUNNER_CACHE[key]


def prep_inputs(mask, query, proj_key, value, Wq, w_energy, c_pad=0):
    """Host-side input prep + per-core sharding. Returns in_maps list."""
    mask = np.asarray(mask)
    query = np.asarray(query, dtype=np.float32)
    proj_key = np.asarray(proj_key, dtype=np.float32)
    value = np.asarray(value, dtype=np.float32)
    wq_t = np.ascontiguousarray(np.asarray(Wq, dtype=np.float32).T)     # [Q, H]
    w_energy = np.ascontiguousarray(np.asarray(w_energy, dtype=np.float32))
    sparse = c_pad > 0
    n_chunk = (c_pad // 128) if sparse else N_CHUNK
    in_maps = []
    for core in range(N_CORES):
        bs = slice(core * B_LOC, (core + 1) * B_LOC)
        qc = query[bs, 0, :].reshape(B_LOC, Q // 128, 128)
        query_pt = np.ascontiguousarray(np.transpose(qc, (2, 1, 0))
                                        ).reshape(128, (Q // 128) * B_LOC)
        m = {
            "proj_key": np.ascontiguousarray(proj_key[bs]),
            "value": np.ascontiguousarray(value[bs]),
            "wq_t": wq_t,
            "query_pt": query_pt,
            "w_energy": w_energy,
        }
        if sparse:
            mbias = np.full((B_LOC, 128, n_chunk), NEG, dtype=np.float32)
            gidx = np.zeros((B_LOC, 128, c_pad // 16), dtype=np.int16)
            for b in range(B_LOC):
                rows = np.nonzero(mask[core * B_LOC + b, 0] != 0)[0]
                cnt = len(rows)
                assert cnt <= c_pad, f"c_pad {c_pad} < count {cnt}"
                padded = np.zeros(c_pad, dtype=np.int16)
                padded[:cnt] = rows
                # pad slots repeat row 0 (always transferred; bias kills them)
                wrapped = padded.reshape(c_pad // 16, 16).T     # [16, c_pad/16]
                # replicated into every 16-partition group (one per Q7 core)
                gidx[b] = np.tile(wrapped, (8, 1))
                mobias = np.full(c_pad, NEG, dtype=np.float32)
                mobias[:cnt] = 0.0
                mbias[b] = mobias.reshape(n_chunk, 128).T       # [128, n_chunk]
            m["mask_bias"] = mbias
            m["gather_idx"] = gidx
        else:
            mm = mask[bs, 0, :].reshape(B_LOC, N_CHUNK, 128)
            mbias = np.where(np.transpose(mm, (0, 2, 1)) == 0, NEG, 0.0
                             ).astype(np.float32)               # [B_LOC,128,n_chunk]
            m["mask_bias"] = np.ascontiguousarray(mbias)
        in_maps.append(m)
    return in_maps


def assemble_outputs(results, mask, c_pad=0):
    """results: per-core dicts with ctx [B_LOC,KV], alphas_c [B_LOC,128,n_chunk]."""
    context = np.zeros((B, 1, KV), dtype=np.float32)
    alphas = np.zeros((B, 1, S), dtype=np.float32)
    sparse = c_pad > 0
    for core in range(N_CORES):
        r = results[core]
        for b in range(B_LOC):
            gb = core * B_LOC + b
            context[gb, 0, :] = r["ctx"][b]
            a = r["alphas_c"][b]                   # [128, n_chunk]
            flat = a.T.reshape(-1)                 # slot i = a[i%128, i//128]
            if sparse:
                rows = np.nonzero(np.asarray(mask)[gb, 0] != 0)[0]
                alphas[gb, 0, rows] = flat[:len(rows)]
            else:
                alphas[gb, 0, :] = flat
    return context, alphas


def pick_c_pad(mask):
    """Smallest multiple of 512 >= max unmasked count over all batches."""
    counts = (np.asarray(mask)[:, 0, :] != 0).sum(axis=1)
    cmax = int(counts.max())
    return ((cmax + 511) // 512) * 512


# Default mode for grading: sparse gather if it fits, else dense.
KERNEL_MODE = os.environ.get("ATT_KERNEL_MODE", "sparse")


def kernel(mask, query, proj_key, value, Wq, w_energy):
    c_pads = []
    if KERNEL_MODE == "sparse":
        c_pads.append(pick_c_pad(mask))
    c_pads.append(0)          # dense fallback
    last_err = None
    for c_pad in c_pads:
        try:
            run = get_runner(k_iters=1, f32r=True, c_pad=c_pad)
            in_maps = prep_inputs(mask, query, proj_key, value, Wq, w_energy,
                                  c_pad=c_pad)
            results, _ = run(in_maps, reps=1)
            return assemble_outputs(results, mask, c_pad=c_pad)
        except Exception as e:          # fall back to dense on any failure
            last_err = e
    raise last_err


# revision 13
# speedup vs baseline: 1.6516x; 1.6516x over previous
"""Trainium2 Bass kernel for nn_Attention_82867099009484 (sparse_attention).

Reference computation (B=16, S=4096, H=1024, Q=1024, K=2048):
    q      = query @ Wq.T                      [B,1,H]
    scores = tanh(q + proj_key) . w_energy     [B,S]
    scores = where(mask==0, -inf, scores)
    alphas = softmax(scores)                   [B,1,S]
    ctx    = alphas @ value                    [B,1,K]
    return (ctx, alphas)

Strategy: data-parallel over batch across 8 NeuronCores (2 batches/core).
Per core, stream proj_key/value row-chunks of 128 seq positions with S on
SBUF partitions:
  - DVE adds broadcast q, ScalarE computes tanh, DVE tensor_tensor_reduce
    fuses the w_energy multiply + h-reduction into one pass -> scores.
  - Softmax without max-subtraction (scores bounded by sum|w| ~ 16, exp
    cannot overflow f32); the mask enters as an additive bias (0 / -1e30)
    folded into the Exp activation's per-partition bias operand.
  - TensorE accumulates ctx = sum_s exp_s * value_s into PSUM chunk by
    chunk (unnormalized); everything is scaled by 1/total at the end.
Host side only reshapes/transposes small tensors (Wq 4MB, query 64KB,
mask 256KB) and assembles the sharded outputs.

Sparse mode: mask sparsity (~50%) means masked rows contribute nothing.
Host computes per-batch lists of unmasked row indices; the device gathers
only those rows of proj_key/value via dma_gather, halving HBM traffic.
"""
import sys, os, time

for _p in ("/opt/trn_rl_repo", "/root/.axon_site/_ro/trn_rl_repo"):
    if os.path.isdir(_p) and _p not in sys.path:
        sys.path.append(_p)

import numpy as np

B, S, H, Q, KV = 16, 4096, 1024, 1024, 2048
N_CORES = 8
B_LOC = B // N_CORES          # 2 batches per core
N_CHUNK = S // 128            # 32 chunks of 128 seq rows (dense)
NEG = -1.0e30

_RUNNER_CACHE = {}


def _build(cfg):
    """Build + bacc-compile the Bass graph. cfg keys: k_iters, f32r, c_pad."""
    import concourse.bacc as bacc
    import concourse.tile as tile
    import concourse.bass as bass
    from concourse import mybir

    F32 = mybir.dt.float32
    F32R = mybir.dt.float32r
    I16 = mybir.dt.int16
    AF = mybir.ActivationFunctionType
    ALU = mybir.AluOpType
    AX = mybir.AxisListType

    k_iters = cfg["k_iters"]
    mm_dt = F32R if cfg["f32r"] else F32
    c_pad = cfg["c_pad"]          # 0 => dense; else padded gather count
    sparse = c_pad > 0
    n_chunk = (c_pad // 128) if sparse else N_CHUNK
    G_ROWS = 512                  # rows per dma_gather instruction
    assert not sparse or c_pad % G_ROWS == 0

    nc = bacc.Bacc("TRN2", target_bir_lowering=False, num_devices=N_CORES)

    pk = nc.dram_tensor("proj_key", [B_LOC, S, H], F32, kind="ExternalInput").ap()
    val = nc.dram_tensor("value", [B_LOC, S, KV], mm_dt, kind="ExternalInput").ap()
    wqt = nc.dram_tensor("wq_t", [Q, H], mm_dt, kind="ExternalInput").ap()
    qpt = nc.dram_tensor("query_pt", [128, (Q // 128) * B_LOC], mm_dt,
                         kind="ExternalInput").ap()
    wen = nc.dram_tensor("w_energy", [H], F32, kind="ExternalInput").ap()
    mb = nc.dram_tensor("mask_bias", [B_LOC, 128, n_chunk], F32,
                        kind="ExternalInput").ap()
    if sparse:
        idx = nc.dram_tensor("gather_idx", [B_LOC, 128, c_pad // 16], I16,
                             kind="ExternalInput").ap()
    ctx_o = nc.dram_tensor("ctx", [B_LOC, KV], F32, kind="ExternalOutput").ap()
    al_o = nc.dram_tensor("alphas_c", [B_LOC, 128, n_chunk], F32,
                          kind="ExternalOutput").ap()

    with tile.TileContext(nc) as tc:
        from contextlib import ExitStack
        with ExitStack() as ctx:
            const = ctx.enter_context(tc.tile_pool(name="const", bufs=1))
            dma_bufs = 2 if sparse else 3
            pkp = ctx.enter_context(tc.tile_pool(name="pkp", bufs=dma_bufs))
            vp = ctx.enter_context(tc.tile_pool(name="vp", bufs=dma_bufs))
            sump = ctx.enter_context(tc.tile_pool(name="sump", bufs=3))
            tanhp = ctx.enter_context(tc.tile_pool(name="tanhp", bufs=3))
            smalls = ctx.enter_context(tc.tile_pool(name="smalls", bufs=2))
            outp = ctx.enter_context(tc.tile_pool(name="outp", bufs=2))
            psq = ctx.enter_context(tc.tile_pool(name="psq", bufs=1, space="PSUM"))
            pstot = ctx.enter_context(tc.tile_pool(name="pstot", bufs=1, space="PSUM"))
            psctx = ctx.enter_context(tc.tile_pool(name="psctx", bufs=1, space="PSUM"))

            # ---- constants (loaded once, reused across iterations) ----
            wqt_sb = const.tile([128, Q // 128, H], mm_dt)   # (p, t, h) = wq_t[t*128+p, h]
            nc.sync.dma_start(wqt_sb[:], wqt.rearrange("(t p) h -> p t h", p=128))
            qpt_sb = const.tile([128, (Q // 128) * B_LOC], mm_dt)
            nc.sync.dma_start(qpt_sb[:], qpt[:])
            wb_sb = const.tile([128, H], F32)                # w_energy bcast over partitions
            nc.gpsimd.dma_start(
                out=wb_sb[:],
                in_=bass.AP(tensor=wen.tensor, offset=wen.offset,
                            ap=[[0, 128]] + list(wen.ap)))
            ones_sb = const.tile([128, 128], F32)
            nc.vector.memset(ones_sb[:], 1.0)
            ones1_sb = const.tile([1, 128], F32)
            nc.vector.memset(ones1_sb[:], 1.0)
            mb_sb = const.tile([128, B_LOC, n_chunk], F32)
            for b in range(B_LOC):
                nc.sync.dma_start(mb_sb[:, b, :], mb[b])
            if sparse:
                idx_sb = const.tile([128, B_LOC, c_pad // 16], I16)
                for b in range(B_LOC):
                    nc.sync.dma_start(idx_sb[:, b, :], idx[b])

            for it in range(k_iters):
                for b in range(B_LOC):
                    # ---- q = query[b] @ Wq.T, as [1, H] in PSUM ----
                    ps_q = psq.tile([1, H], F32, tag="psq_shared")
                    for hc in range(H // 512):
                        for t in range(Q // 128):
                            nc.tensor.matmul(
                                ps_q[0:1, hc * 512:(hc + 1) * 512],
                                lhsT=qpt_sb[:, (t * B_LOC + b):(t * B_LOC + b + 1)],
                                rhs=wqt_sb[:, t, hc * 512:(hc + 1) * 512],
                                start=(t == 0), stop=(t == Q // 128 - 1))
                    q_sb = smalls.tile([1, H], F32)
                    nc.vector.tensor_copy(q_sb[:], ps_q[:])
                    # broadcast across partitions: ones[1,128].T @ q_sb[1,H]
                    ps_qb = psq.tile([128, H], F32, tag="psq_shared")
                    for kk in range(H // 512):
                        nc.tensor.matmul(ps_qb[:, kk * 512:(kk + 1) * 512],
                                         lhsT=ones1_sb[:],
                                         rhs=q_sb[:, kk * 512:(kk + 1) * 512],
                                         start=True, stop=True)
                    qb_sb = smalls.tile([128, H], F32)
                    nc.vector.tensor_copy(qb_sb[:], ps_qb[:])

                    exp_sb = smalls.tile([128, n_chunk], mm_dt)
                    scores_sb = smalls.tile([128, n_chunk], F32)
                    ps_ctx = psctx.tile([1, KV], F32)

                    if sparse:
                        # gather unmasked rows, G_ROWS per dma_gather
                        pk_g = []
                        v_g = []
                        for g in range(c_pad // G_ROWS):
                            pkt = pkp.tile([128, G_ROWS // 128, H], F32)
                            nc.gpsimd.dma_gather(
                                out_ap=pkt[:], in_ap=pk[b],
                                idxs_ap=idx_sb[:, b, g * (G_ROWS // 16):(g + 1) * (G_ROWS // 16)],
                                num_idxs=G_ROWS, num_idxs_reg=G_ROWS,
                                elem_size=H, elem_step=H)
                            vt = vp.tile([128, G_ROWS // 128, KV], mm_dt)
                            nc.gpsimd.dma_gather(
                                out_ap=vt[:], in_ap=val[b],
                                idxs_ap=idx_sb[:, b, g * (G_ROWS // 16):(g + 1) * (G_ROWS // 16)],
                                num_idxs=G_ROWS, num_idxs_reg=G_ROWS,
                                elem_size=KV, elem_step=KV)
                            pk_g.append(pkt)
                            v_g.append(vt)

                    for c in range(n_chunk):
                        if sparse:
                            pk_t = pk_g[c // (G_ROWS // 128)][:, c % (G_ROWS // 128), :]
                            v_t = v_g[c // (G_ROWS // 128)][:, c % (G_ROWS // 128), :]
                        else:
                            pk_t_full = pkp.tile([128, H], F32)
                            nc.sync.dma_start(pk_t_full[:], pk[b, c * 128:(c + 1) * 128, :])
                            v_t_full = vp.tile([128, KV], mm_dt)
                            nc.scalar.dma_start(v_t_full[:], val[b, c * 128:(c + 1) * 128, :])
                            pk_t = pk_t_full[:]
                            v_t = v_t_full[:]

                        sum_t = sump.tile([128, H], F32)
                        nc.vector.tensor_add(sum_t[:], pk_t, qb_sb[:])
                        tanh_t = tanhp.tile([128, H], F32)
                        nc.scalar.activation(tanh_t[:], sum_t[:], AF.Tanh)
                        # sum_t reused as dummy elementwise output;
                        # accum_out = sum_h(tanh * w_energy) in one DVE pass
                        nc.vector.scalar_tensor_tensor(
                            out=sum_t[:], in0=tanh_t[:], scalar=1.0,
                            in1=wb_sb[:], op0=ALU.mult, op1=ALU.mult,
                            accum_out=scores_sb[:, c:c + 1])
                        # exp(scores + mask_bias); masked/pad rows -> exp(-1e30) = 0
                        nc.scalar.activation(exp_sb[:, c:c + 1], scores_sb[:, c:c + 1],
                                             AF.Exp, bias=mb_sb[:, b, c:c + 1])
                        for k in range(KV // 512):
                            nc.tensor.matmul(
                                ps_ctx[0:1, k * 512:(k + 1) * 512],
                                lhsT=exp_sb[:, c:c + 1],
                                rhs=v_t[:, k * 512:(k + 1) * 512],
                                start=(c == 0), stop=(c == n_chunk - 1))

                    # ---- softmax denominator + normalization ----
                    sums = smalls.tile([128, 1], F32)
                    exp_f32 = exp_sb[:].bitcast(F32)
                    nc.vector.reduce_sum(sums[:], exp_f32, axis=AX.X)
                    ps_tot = pstot.tile([128, 1], F32)
                    nc.tensor.matmul(ps_tot[:], lhsT=ones_sb[:], rhs=sums[:],
                                     start=True, stop=True)
                    inv_sb = smalls.tile([128, 1], F32)
                    nc.vector.reciprocal(inv_sb[:], ps_tot[:])
                    al_sb = outp.tile([128, n_chunk], F32)
                    nc.vector.tensor_scalar_mul(al_sb[:], exp_f32, inv_sb[:, 0:1])
                    nc.sync.dma_start(al_o[b], al_sb[:])
                    for k in range(KV // 512):
                        ctx_sb = outp.tile([1, 512], F32)
                        nc.vector.tensor_scalar_mul(
                            ctx_sb[:], ps_ctx[0:1, k * 512:(k + 1) * 512],
                            inv_sb[0:1, 0:1])
                        nc.sync.dma_start(ctx_o[b:b + 1, k * 512:(k + 1) * 512],
                                          ctx_sb[:])

    nc.compile()
    return nc


def _make_runner(cfg):
    """Compile once; return f(in_maps) -> (fn, dargs-builder) reusable callable."""
    import jax
    from jax.sharding import Mesh, PartitionSpec
    from jax.experimental.shard_map import shard_map
    from concourse import mybir
    from concourse.bass2jax import (_bass_exec_p, partition_id_tensor,
                                    install_neuronx_cc_hook)

    nc = _build(cfg)
    install_neuronx_cc_hook()
    partition_name = nc.partition_id_tensor.name if nc.partition_id_tensor else None
    in_names, out_names, out_avals, zero_outs = [], [], [], []
    for alloc in nc.m.functions[0].allocations:
        if not isinstance(alloc, mybir.MemoryLocationSet):
            continue
        name = alloc.memorylocations[0].name
        if alloc.kind == "ExternalInput":
            if name != partition_name:
                in_names.append(name)
        elif alloc.kind == "ExternalOutput":
            out_names.append(name)
            out_avals.append(jax.core.ShapedArray(
                tuple(alloc.tensor_shape), mybir.dt.np(alloc.dtype)))
            zero_outs.append(np.zeros(tuple(alloc.tensor_shape),
                                      mybir.dt.np(alloc.dtype)))
    n_params = len(in_names)
    all_in = list(in_names) + list(out_names) + (
        [partition_name] if partition_name else [])

    def _body(*args):
        ops = list(args)
        if partition_name:
            ops.append(partition_id_tensor())
        return tuple(_bass_exec_p.bind(
            *ops, out_avals=tuple(out_avals), in_names=tuple(all_in),
            out_names=tuple(out_names), lowering_input_output_aliases=(),
            sim_require_finite=True, sim_require_nnan=True, nc=nc))

    import jax
    devices = jax.devices()[:N_CORES]
    mesh = Mesh(np.asarray(devices), ("core",))
    nio = n_params + len(out_names)
    fn = jax.jit(shard_map(_body, mesh=mesh,
                           in_specs=(PartitionSpec("core"),) * nio,
                           out_specs=(PartitionSpec("core"),) * len(out_names),
                           check_rep=False))

    def _stage(in_maps):
        from jax.sharding import NamedSharding
        sh = NamedSharding(mesh, PartitionSpec("core"))

        def to_global(per_core_arrs):
            """Build a sharded global array from per-device shards without
            going through jax's _multi_slice path."""
            shards = [jax.device_put(a, d)
                      for a, d in zip(per_core_arrs, devices)]
            shape = (sum(a.shape[0] for a in per_core_arrs),) + per_core_arrs[0].shape[1:]
            return jax.make_array_from_single_device_arrays(shape, sh, shards)

        dargs = []
        for nm in in_names:
            dargs.append(to_global([np.asarray(m[nm]) for m in in_maps]))
        for z in zero_outs:
            dargs.append(to_global([z] * N_CORES))
        jax.block_until_ready(dargs)
        return dargs

    def make_call(in_maps):
        dargs = _stage(in_maps)

        def call():
            t0 = time.perf_counter()
            outs = fn(*dargs)
            jax.block_until_ready(outs)
            return time.perf_counter() - t0, outs
        return call

    def run(in_maps, reps=1):
        call = make_call(in_maps)
        call()                               # warm call
        times = []
        for _ in range(reps):
            dt, outs = call()
            times.append(dt)
        res = []
        for c in range(N_CORES):
            m = {}
            for i, nm in enumerate(out_names):
                arr = np.asarray(outs[i])
                per = arr.shape[0] // N_CORES
                m[nm] = arr[c * per:(c + 1) * per]
            res.append(m)
        return res, times

    run.make_call = make_call
    return run


def get_runner(k_iters=1, f32r=True, c_pad=0):
    key = (k_iters, f32r, c_pad)
    if key not in _RUNNER_CACHE:
        _RUNNER_CACHE[key] = _make_runner(
            {"k_iters": k_iters, "f32r": f32r, "c_pad": c_pad})
    return _RUNNER_CACHE[key]


def prep_inputs(mask, query, proj_key, value, Wq, w_energy, c_pad=0):
    """Host-side input prep + per-core sharding. Returns in_maps list."""
    mask = np.asarray(mask)
    query = np.asarray(query, dtype=np.float32)
    proj_key = np.asarray(proj_key, dtype=np.float32)
    value = np.asarray(value, dtype=np.float32)
    wq_t = np.ascontiguousarray(np.asarray(Wq, dtype=np.float32).T)     # [Q, H]
    w_energy = np.ascontiguousarray(np.asarray(w_energy, dtype=np.float32))
    sparse = c_pad > 0
    n_chunk = (c_pad // 128) if sparse else N_CHUNK
    in_maps = []
    for core in range(N_CORES):
        bs = slice(core * B_LOC, (core + 1) * B_LOC)
        qc = query[bs, 0, :].reshape(B_LOC, Q // 128, 128)
        query_pt = np.ascontiguousarray(np.transpose(qc, (2, 1, 0))
                                        ).reshape(128, (Q // 128) * B_LOC)
        m = {
            "proj_key": np.ascontiguousarray(proj_key[bs]),
            "value": np.ascontiguousarray(value[bs]),
            "wq_t": wq_t,
            "query_pt": query_pt,
            "w_energy": w_energy,
        }
        if sparse:
            mbias = np.full((B_LOC, 128, n_chunk), NEG, dtype=np.float32)
            gidx = np.zeros((B_LOC, 128, c_pad // 16), dtype=np.int16)
            for b in range(B_LOC):
                rows = np.nonzero(mask[core * B_LOC + b, 0] != 0)[0]
                cnt = len(rows)
                assert cnt <= c_pad, f"c_pad {c_pad} < count {cnt}"
                padded = np.zeros(c_pad, dtype=np.int16)
                padded[:cnt] = rows
                # pad slots repeat row 0 (always transferred; bias kills them)
                wrapped = padded.reshape(c_pad // 16, 16).T     # [16, c_pad/16]
                # replicated into every 16-partition group (one per Q7 core)
                gidx[b] = np.tile(wrapped, (8, 1))
                mobias = np.full(c_pad, NEG, dtype=np.float32)
                mobias[:cnt] = 0.0
                mbias[b] = mobias.reshape(n_chunk, 128).T       # [128, n_chunk]
            m["mask_bias"] = mbias
            m["gather_idx"] = gidx
        else:
            mm = mask[bs, 0, :].reshape(B_LOC, N_CHUNK, 128)
            mbias = np.where(np.transpose(mm, (0, 2, 1)) == 0, NEG, 0.0
                             ).astype(np.float32)               # [B_LOC,128,n_chunk]
            m["mask_bias"] = np.ascontiguousarray(mbias)
        in_maps.append(m)
    return in_maps


def assemble_outputs(results, mask, c_pad=0):
    """results: per-core dicts with ctx [B_LOC,KV], alphas_c [B_LOC,128,n_chunk]."""
    context = np.zeros((B, 1, KV), dtype=np.float32)
    alphas = np.zeros((B, 1, S), dtype=np.float32)
    sparse = c_pad > 0
    for core in range(N_CORES):
        r = results[core]
        for b in range(B_LOC):
            gb = core * B_LOC + b
            context[gb, 0, :] = r["ctx"][b]
            a = r["alphas_c"][b]                   # [128, n_chunk]
            flat = a.T.reshape(-1)                 # slot i = a[i%128, i//128]
            if sparse:
                rows = np.nonzero(np.asarray(mask)[gb, 0] != 0)[0]
                alphas[gb, 0, rows] = flat[:len(rows)]
            else:
                alphas[gb, 0, :] = flat
    return context, alphas


def pick_c_pad(mask):
    """Smallest multiple of 512 >= max unmasked count over all batches."""
    counts = (np.asarray(mask)[:, 0, :] != 0).sum(axis=1)
    cmax = int(counts.max())
    return ((cmax + 511) // 512) * 512


# Default mode for grading: sparse gather if it fits, else dense.
KERNEL_MODE = os.environ.get("ATT_KERNEL_MODE", "sparse")


def kernel(mask, query, proj_key, value, Wq, w_energy):
    c_pads = []
    if KERNEL_MODE == "sparse":
        c_pads.append(pick_c_pad(mask))
    c_pads.append(0)          # dense fallback
    last_err = None
    for c_pad in c_pads:
        try:
            run = get_runner(k_iters=1, f32r=True, c_pad=c_pad)
            in_maps = prep_inputs(mask, query, proj_key, value, Wq, w_energy,
                                  c_pad=c_pad)
            results, _ = run(in_maps, reps=1)
            return assemble_outputs(results, mask, c_pad=c_pad)
        except Exception as e:          # fall back to dense on any failure
            last_err = e
    raise last_err


# revision 14
# speedup vs baseline: 2.0409x; 1.2357x over previous
"""Trainium2 Bass kernel for nn_Attention_82867099009484 (sparse_attention).

Reference computation (B=16, S=4096, H=1024, Q=1024, K=2048):
    q      = query @ Wq.T                      [B,1,H]
    scores = tanh(q + proj_key) . w_energy     [B,S]
    scores = where(mask==0, -inf, scores)
    alphas = softmax(scores)                   [B,1,S]
    ctx    = alphas @ value                    [B,1,K]
    return (ctx, alphas)

Strategy: data-parallel over batch across 8 NeuronCores (2 batches/core).
Per core, stream proj_key/value row-chunks of 128 seq positions with S on
SBUF partitions:
  - DVE adds broadcast q, ScalarE computes tanh, DVE tensor_tensor_reduce
    fuses the w_energy multiply + h-reduction into one pass -> scores.
  - Softmax without max-subtraction (scores bounded by sum|w| ~ 16, exp
    cannot overflow f32); the mask enters as an additive bias (0 / -1e30)
    folded into the Exp activation's per-partition bias operand.
  - TensorE accumulates ctx = sum_s exp_s * value_s into PSUM chunk by
    chunk (unnormalized); everything is scaled by 1/total at the end.
Host side only reshapes/transposes small tensors (Wq 4MB, query 64KB,
mask 256KB) and assembles the sharded outputs.

Sparse mode: mask sparsity (~50%) means masked rows contribute nothing.
Host computes per-batch lists of unmasked row indices; the device gathers
only those rows of proj_key/value via dma_gather, halving HBM traffic.
"""
import sys, os, time

for _p in ("/opt/trn_rl_repo", "/root/.axon_site/_ro/trn_rl_repo"):
    if os.path.isdir(_p) and _p not in sys.path:
        sys.path.append(_p)

import numpy as np

B, S, H, Q, KV = 16, 4096, 1024, 1024, 2048
N_CORES = 8
B_LOC = B // N_CORES          # 2 batches per core
N_CHUNK = S // 128            # 32 chunks of 128 seq rows (dense)
NEG = -1.0e30

_RUNNER_CACHE = {}


def _build(cfg):
    """Build + bacc-compile the Bass graph. cfg keys: k_iters, f32r, c_pad."""
    import concourse.bacc as bacc
    import concourse.tile as tile
    import concourse.bass as bass
    from concourse import mybir

    F32 = mybir.dt.float32
    F32R = mybir.dt.float32r
    I16 = mybir.dt.int16
    AF = mybir.ActivationFunctionType
    ALU = mybir.AluOpType
    AX = mybir.AxisListType

    k_iters = cfg["k_iters"]
    mm_dt = F32R if cfg["f32r"] else F32
    c_pad = cfg["c_pad"]          # 0 => dense; else padded gather count
    sparse = c_pad > 0
    n_chunk = (c_pad // 128) if sparse else N_CHUNK
    G_ROWS = 512                  # max rows per dma_gather instruction
    assert not sparse or c_pad % 128 == 0
    if sparse:
        g_sizes = [G_ROWS] * (c_pad // G_ROWS)
        if c_pad % G_ROWS:
            g_sizes.append(c_pad % G_ROWS)
    else:
        g_sizes = []

    nc = bacc.Bacc("TRN2", target_bir_lowering=False, num_devices=N_CORES)

    pk = nc.dram_tensor("proj_key", [B_LOC, S, H], F32, kind="ExternalInput").ap()
    val = nc.dram_tensor("value", [B_LOC, S, KV], mm_dt, kind="ExternalInput").ap()
    wqt = nc.dram_tensor("wq_t", [Q, H], mm_dt, kind="ExternalInput").ap()
    qpt = nc.dram_tensor("query_pt", [128, (Q // 128) * B_LOC], mm_dt,
                         kind="ExternalInput").ap()
    wen = nc.dram_tensor("w_energy", [H], F32, kind="ExternalInput").ap()
    mb = nc.dram_tensor("mask_bias", [B_LOC, 128, n_chunk], F32,
                        kind="ExternalInput").ap()
    if sparse:
        idx = nc.dram_tensor("gather_idx", [B_LOC, 128, c_pad // 16], I16,
                             kind="ExternalInput").ap()
    ctx_o = nc.dram_tensor("ctx", [B_LOC, KV], F32, kind="ExternalOutput").ap()
    al_o = nc.dram_tensor("alphas_c", [B_LOC, 128, n_chunk], F32,
                          kind="ExternalOutput").ap()

    with tile.TileContext(nc) as tc:
        from contextlib import ExitStack
        with ExitStack() as ctx:
            const = ctx.enter_context(tc.tile_pool(name="const", bufs=1))
            dma_bufs = 2 if sparse else 3
            pkp = ctx.enter_context(tc.tile_pool(name="pkp", bufs=dma_bufs))
            vp = ctx.enter_context(tc.tile_pool(name="vp", bufs=dma_bufs))
            sump = ctx.enter_context(tc.tile_pool(name="sump", bufs=3))
            tanhp = ctx.enter_context(tc.tile_pool(name="tanhp", bufs=3))
            smalls = ctx.enter_context(tc.tile_pool(name="smalls", bufs=2))
            outp = ctx.enter_context(tc.tile_pool(name="outp", bufs=2))
            psq = ctx.enter_context(tc.tile_pool(name="psq", bufs=1, space="PSUM"))
            pstot = ctx.enter_context(tc.tile_pool(name="pstot", bufs=1, space="PSUM"))
            psctx = ctx.enter_context(tc.tile_pool(name="psctx", bufs=1, space="PSUM"))

            # ---- constants (loaded once, reused across iterations) ----
            wqt_sb = const.tile([128, Q // 128, H], mm_dt)   # (p, t, h) = wq_t[t*128+p, h]
            nc.sync.dma_start(wqt_sb[:], wqt.rearrange("(t p) h -> p t h", p=128))
            qpt_sb = const.tile([128, (Q // 128) * B_LOC], mm_dt)
            nc.sync.dma_start(qpt_sb[:], qpt[:])
            wb_sb = const.tile([128, H], F32)                # w_energy bcast over partitions
            nc.gpsimd.dma_start(
                out=wb_sb[:],
                in_=bass.AP(tensor=wen.tensor, offset=wen.offset,
                            ap=[[0, 128]] + list(wen.ap)))
            ones_sb = const.tile([128, 128], F32)
            nc.vector.memset(ones_sb[:], 1.0)
            ones1_sb = const.tile([1, 128], F32)
            nc.vector.memset(ones1_sb[:], 1.0)
            mb_sb = const.tile([128, B_LOC, n_chunk], F32)
            for b in range(B_LOC):
                nc.sync.dma_start(mb_sb[:, b, :], mb[b])
            if sparse:
                idx_sb = const.tile([128, B_LOC, c_pad // 16], I16)
                for b in range(B_LOC):
                    nc.sync.dma_start(idx_sb[:, b, :], idx[b])

            for it in range(k_iters):
                for b in range(B_LOC):
                    # ---- q = query[b] @ Wq.T, as [1, H] in PSUM ----
                    ps_q = psq.tile([1, H], F32, tag="psq_shared")
                    for hc in range(H // 512):
                        for t in range(Q // 128):
                            nc.tensor.matmul(
                                ps_q[0:1, hc * 512:(hc + 1) * 512],
                                lhsT=qpt_sb[:, (t * B_LOC + b):(t * B_LOC + b + 1)],
                                rhs=wqt_sb[:, t, hc * 512:(hc + 1) * 512],
                                start=(t == 0), stop=(t == Q // 128 - 1))
                    q_sb = smalls.tile([1, H], F32)
                    nc.vector.tensor_copy(q_sb[:], ps_q[:])
                    # broadcast across partitions: ones[1,128].T @ q_sb[1,H]
                    ps_qb = psq.tile([128, H], F32, tag="psq_shared")
                    for kk in range(H // 512):
                        nc.tensor.matmul(ps_qb[:, kk * 512:(kk + 1) * 512],
                                         lhsT=ones1_sb[:],
                                         rhs=q_sb[:, kk * 512:(kk + 1) * 512],
                                         start=True, stop=True)
                    qb_sb = smalls.tile([128, H], F32)
                    nc.vector.tensor_copy(qb_sb[:], ps_qb[:])

                    exp_sb = smalls.tile([128, n_chunk], mm_dt)
                    scores_sb = smalls.tile([128, n_chunk], F32)
                    ps_ctx = psctx.tile([1, KV], F32)

                    if sparse:
                        # gather unmasked rows, up to G_ROWS per dma_gather
                        pk_g = []
                        v_g = []
                        goff = 0
                        for gs in g_sizes:
                            i0, i1 = goff // 16, (goff + gs) // 16
                            pkt = pkp.tile([128, G_ROWS // 128, H], F32,
                                           tag="pk_gather")
                            nc.gpsimd.dma_gather(
                                out_ap=pkt[:, 0:gs // 128, :], in_ap=pk[b],
                                idxs_ap=idx_sb[:, b, i0:i1],
                                num_idxs=gs, num_idxs_reg=gs,
                                elem_size=H, elem_step=H)
                            vt = vp.tile([128, G_ROWS // 128, KV], mm_dt,
                                         tag="v_gather")
                            nc.gpsimd.dma_gather(
                                out_ap=vt[:, 0:gs // 128, :], in_ap=val[b],
                                idxs_ap=idx_sb[:, b, i0:i1],
                                num_idxs=gs, num_idxs_reg=gs,
                                elem_size=KV, elem_step=KV)
                            for cc in range(gs // 128):
                                pk_g.append(pkt[:, cc, :])
                                v_g.append(vt[:, cc, :])
                            goff += gs

                    for c in range(n_chunk):
                        if sparse:
                            pk_t = pk_g[c]
                            v_t = v_g[c]
                        else:
                            pk_t_full = pkp.tile([128, H], F32)
                            nc.sync.dma_start(pk_t_full[:], pk[b, c * 128:(c + 1) * 128, :])
                            v_t_full = vp.tile([128, KV], mm_dt)
                            nc.scalar.dma_start(v_t_full[:], val[b, c * 128:(c + 1) * 128, :])
                            pk_t = pk_t_full[:]
                            v_t = v_t_full[:]

                        sum_t = sump.tile([128, H], F32)
                        nc.vector.tensor_add(sum_t[:], pk_t, qb_sb[:])
                        tanh_t = tanhp.tile([128, H], F32)
                        nc.scalar.activation(tanh_t[:], sum_t[:], AF.Tanh)
                        # sum_t reused as dummy elementwise output;
                        # accum_out = sum_h(tanh * w_energy) in one DVE pass
                        nc.vector.scalar_tensor_tensor(
                            out=sum_t[:], in0=tanh_t[:], scalar=1.0,
                            in1=wb_sb[:], op0=ALU.mult, op1=ALU.mult,
                            accum_out=scores_sb[:, c:c + 1])
                        # exp(scores + mask_bias); masked/pad rows -> exp(-1e30) = 0
                        nc.scalar.activation(exp_sb[:, c:c + 1], scores_sb[:, c:c + 1],
                                             AF.Exp, bias=mb_sb[:, b, c:c + 1])
                        for k in range(KV // 512):
                            nc.tensor.matmul(
                                ps_ctx[0:1, k * 512:(k + 1) * 512],
                                lhsT=exp_sb[:, c:c + 1],
                                rhs=v_t[:, k * 512:(k + 1) * 512],
                                start=(c == 0), stop=(c == n_chunk - 1))

                    # ---- softmax denominator + normalization ----
                    sums = smalls.tile([128, 1], F32)
                    exp_f32 = exp_sb[:].bitcast(F32)
                    nc.vector.reduce_sum(sums[:], exp_f32, axis=AX.X)
                    ps_tot = pstot.tile([128, 1], F32)
                    nc.tensor.matmul(ps_tot[:], lhsT=ones_sb[:], rhs=sums[:],
                                     start=True, stop=True)
                    inv_sb = smalls.tile([128, 1], F32)
                    nc.vector.reciprocal(inv_sb[:], ps_tot[:])
                    al_sb = outp.tile([128, n_chunk], F32)
                    nc.vector.tensor_scalar_mul(al_sb[:], exp_f32, inv_sb[:, 0:1])
                    nc.sync.dma_start(al_o[b], al_sb[:])
                    for k in range(KV // 512):
                        ctx_sb = outp.tile([1, 512], F32)
                        nc.vector.tensor_scalar_mul(
                            ctx_sb[:], ps_ctx[0:1, k * 512:(k + 1) * 512],
                            inv_sb[0:1, 0:1])
                        nc.sync.dma_start(ctx_o[b:b + 1, k * 512:(k + 1) * 512],
                                          ctx_sb[:])

    nc.compile()
    return nc


def _make_runner(cfg):
    """Compile once; return f(in_maps) -> (fn, dargs-builder) reusable callable."""
    import jax
    from jax.sharding import Mesh, PartitionSpec
    from jax.experimental.shard_map import shard_map
    from concourse import mybir
    from concourse.bass2jax import (_bass_exec_p, partition_id_tensor,
                                    install_neuronx_cc_hook)

    nc = _build(cfg)
    install_neuronx_cc_hook()
    partition_name = nc.partition_id_tensor.name if nc.partition_id_tensor else None
    in_names, out_names, out_avals, zero_outs = [], [], [], []
    for alloc in nc.m.functions[0].allocations:
        if not isinstance(alloc, mybir.MemoryLocationSet):
            continue
        name = alloc.memorylocations[0].name
        if alloc.kind == "ExternalInput":
            if name != partition_name:
                in_names.append(name)
        elif alloc.kind == "ExternalOutput":
            out_names.append(name)
            out_avals.append(jax.core.ShapedArray(
                tuple(alloc.tensor_shape), mybir.dt.np(alloc.dtype)))
            zero_outs.append(np.zeros(tuple(alloc.tensor_shape),
                                      mybir.dt.np(alloc.dtype)))
    n_params = len(in_names)
    all_in = list(in_names) + list(out_names) + (
        [partition_name] if partition_name else [])

    def _body(*args):
        ops = list(args)
        if partition_name:
            ops.append(partition_id_tensor())
        return tuple(_bass_exec_p.bind(
            *ops, out_avals=tuple(out_avals), in_names=tuple(all_in),
            out_names=tuple(out_names), lowering_input_output_aliases=(),
            sim_require_finite=True, sim_require_nnan=True, nc=nc))

    import jax
    devices = jax.devices()[:N_CORES]
    mesh = Mesh(np.asarray(devices), ("core",))
    nio = n_params + len(out_names)
    fn = jax.jit(shard_map(_body, mesh=mesh,
                           in_specs=(PartitionSpec("core"),) * nio,
                           out_specs=(PartitionSpec("core"),) * len(out_names),
                           check_rep=False))

    def _stage(in_maps):
        from jax.sharding import NamedSharding
        sh = NamedSharding(mesh, PartitionSpec("core"))

        def to_global(per_core_arrs):
            """Build a sharded global array from per-device shards without
            going through jax's _multi_slice path."""
            shards = [jax.device_put(a, d)
                      for a, d in zip(per_core_arrs, devices)]
            shape = (sum(a.shape[0] for a in per_core_arrs),) + per_core_arrs[0].shape[1:]
            return jax.make_array_from_single_device_arrays(shape, sh, shards)

        dargs = []
        for nm in in_names:
            dargs.append(to_global([np.asarray(m[nm]) for m in in_maps]))
        for z in zero_outs:
            dargs.append(to_global([z] * N_CORES))
        jax.block_until_ready(dargs)
        return dargs

    def make_call(in_maps):
        dargs = _stage(in_maps)

        def call():
            t0 = time.perf_counter()
            outs = fn(*dargs)
            jax.block_until_ready(outs)
            return time.perf_counter() - t0, outs
        return call

    def run(in_maps, reps=1):
        call = make_call(in_maps)
        call()                               # warm call
        times = []
        for _ in range(reps):
            dt, outs = call()
            times.append(dt)
        res = []
        for c in range(N_CORES):
            m = {}
            for i, nm in enumerate(out_names):
                arr = np.asarray(outs[i])
                per = arr.shape[0] // N_CORES
                m[nm] = arr[c * per:(c + 1) * per]
            res.append(m)
        return res, times

    run.make_call = make_call
    return run


def get_runner(k_iters=1, f32r=True, c_pad=0):
    key = (k_iters, f32r, c_pad)
    if key not in _RUNNER_CACHE:
        _RUNNER_CACHE[key] = _make_runner(
            {"k_iters": k_iters, "f32r": f32r, "c_pad": c_pad})
    return _RUNNER_CACHE[key]


def prep_inputs(mask, query, proj_key, value, Wq, w_energy, c_pad=0):
    """Host-side input prep + per-core sharding. Returns in_maps list."""
    mask = np.asarray(mask)
    query = np.asarray(query, dtype=np.float32)
    proj_key = np.asarray(proj_key, dtype=np.float32)
    value = np.asarray(value, dtype=np.float32)
    wq_t = np.ascontiguousarray(np.asarray(Wq, dtype=np.float32).T)     # [Q, H]
    w_energy = np.ascontiguousarray(np.asarray(w_energy, dtype=np.float32))
    sparse = c_pad > 0
    n_chunk = (c_pad // 128) if sparse else N_CHUNK
    in_maps = []
    for core in range(N_CORES):
        bs = slice(core * B_LOC, (core + 1) * B_LOC)
        qc = query[bs, 0, :].reshape(B_LOC, Q // 128, 128)
        query_pt = np.ascontiguousarray(np.transpose(qc, (2, 1, 0))
                                        ).reshape(128, (Q // 128) * B_LOC)
        m = {
            "proj_key": np.ascontiguousarray(proj_key[bs]),
            "value": np.ascontiguousarray(value[bs]),
            "wq_t": wq_t,
            "query_pt": query_pt,
            "w_energy": w_energy,
        }
        if sparse:
            mbias = np.full((B_LOC, 128, n_chunk), NEG, dtype=np.float32)
            gidx = np.zeros((B_LOC, 128, c_pad // 16), dtype=np.int16)
            for b in range(B_LOC):
                rows = np.nonzero(mask[core * B_LOC + b, 0] != 0)[0]
                cnt = len(rows)
                assert cnt <= c_pad, f"c_pad {c_pad} < count {cnt}"
                padded = np.zeros(c_pad, dtype=np.int16)
                padded[:cnt] = rows
                # pad slots repeat row 0 (always transferred; bias kills them)
                wrapped = padded.reshape(c_pad // 16, 16).T     # [16, c_pad/16]
                # replicated into every 16-partition group (one per Q7 core)
                gidx[b] = np.tile(wrapped, (8, 1))
                mobias = np.full(c_pad, NEG, dtype=np.float32)
                mobias[:cnt] = 0.0
                mbias[b] = mobias.reshape(n_chunk, 128).T       # [128, n_chunk]
            m["mask_bias"] = mbias
            m["gather_idx"] = gidx
        else:
            mm = mask[bs, 0, :].reshape(B_LOC, N_CHUNK, 128)
            mbias = np.where(np.transpose(mm, (0, 2, 1)) == 0, NEG, 0.0
                             ).astype(np.float32)               # [B_LOC,128,n_chunk]
            m["mask_bias"] = np.ascontiguousarray(mbias)
        in_maps.append(m)
    return in_maps


def assemble_outputs(results, mask, c_pad=0):
    """results: per-core dicts with ctx [B_LOC,KV], alphas_c [B_LOC,128,n_chunk]."""
    context = np.zeros((B, 1, KV), dtype=np.float32)
    alphas = np.zeros((B, 1, S), dtype=np.float32)
    sparse = c_pad > 0
    for core in range(N_CORES):
        r = results[core]
        for b in range(B_LOC):
            gb = core * B_LOC + b
            context[gb, 0, :] = r["ctx"][b]
            a = r["alphas_c"][b]                   # [128, n_chunk]
            flat = a.T.reshape(-1)                 # slot i = a[i%128, i//128]
            if sparse:
                rows = np.nonzero(np.asarray(mask)[gb, 0] != 0)[0]
                alphas[gb, 0, rows] = flat[:len(rows)]
            else:
                alphas[gb, 0, :] = flat
    return context, alphas


def pick_c_pad(mask):
    """Smallest multiple of 128 >= max unmasked count over all batches."""
    counts = (np.asarray(mask)[:, 0, :] != 0).sum(axis=1)
    cmax = int(counts.max())
    return ((cmax + 127) // 128) * 128


# Default mode for grading: sparse gather if it fits, else dense.
KERNEL_MODE = os.environ.get("ATT_KERNEL_MODE", "sparse")


def kernel(mask, query, proj_key, value, Wq, w_energy):
    c_pads = []
    if KERNEL_MODE == "sparse":
        c_pads.append(pick_c_pad(mask))
    c_pads.append(0)          # dense fallback
    last_err = None
    for c_pad in c_pads:
        try:
            run = get_runner(k_iters=1, f32r=True, c_pad=c_pad)
            in_maps = prep_inputs(mask, query, proj_key, value, Wq, w_energy,
                                  c_pad=c_pad)
            results, _ = run(in_maps, reps=1)
            return assemble_outputs(results, mask, c_pad=c_pad)
        except Exception as e:          # fall back to dense on any failure
            last_err = e
    raise last_err


# revision 16
# speedup vs baseline: 2.5785x; 1.2634x over previous
"""Trainium2 Bass kernel for nn_Attention_82867099009484 (sparse_attention).

Reference computation (B=16, S=4096, H=1024, Q=1024, K=2048):
    q      = query @ Wq.T                      [B,1,H]
    scores = tanh(q + proj_key) . w_energy     [B,S]
    scores = where(mask==0, -inf, scores)
    alphas = softmax(scores)                   [B,1,S]
    ctx    = alphas @ value                    [B,1,K]
    return (ctx, alphas)

Strategy: data-parallel over batch across 8 NeuronCores (2 batches/core).
Per core, stream proj_key/value row-chunks of 128 seq positions with S on
SBUF partitions:
  - DVE adds broadcast q, ScalarE computes tanh, DVE scalar_tensor_tensor
    fuses the w_energy multiply + h-reduction into one pass -> scores.
  - Softmax without max-subtraction (scores bounded by sum|w| ~ 16, exp
    cannot overflow f32); the mask enters as an additive bias (0 / -1e30)
    folded into the Exp activation's per-partition bias operand.
  - TensorE accumulates ctx = sum_s exp_s * value_s into PSUM chunk by
    chunk (unnormalized); everything is scaled by 1/total at the end.
Host side only reshapes/transposes small tensors (Wq 4MB, query 64KB,
mask 256KB) and assembles the sharded outputs.

Sparse mode: mask sparsity (~50%) means masked rows contribute nothing.
Host computes per-batch lists of unmasked row indices; the device gathers
only those rows of proj_key/value via dma_gather, halving HBM traffic.
"""
import sys, os, time

for _p in ("/opt/trn_rl_repo", "/root/.axon_site/_ro/trn_rl_repo"):
    if os.path.isdir(_p) and _p not in sys.path:
        sys.path.append(_p)

import numpy as np

B, S, H, Q, KV = 16, 4096, 1024, 1024, 2048
N_CORES = 8
B_LOC = B // N_CORES          # 2 batches per core
N_CHUNK = S // 128            # 32 chunks of 128 seq rows (dense)
NEG = -1.0e30

_RUNNER_CACHE = {}


def _build(cfg):
    """Build + bacc-compile the Bass graph. cfg keys: k_iters, f32r, c_pad."""
    import concourse.bacc as bacc
    import concourse.tile as tile
    import concourse.bass as bass
    from concourse import mybir

    F32 = mybir.dt.float32
    F32R = mybir.dt.float32r
    I16 = mybir.dt.int16
    AF = mybir.ActivationFunctionType
    ALU = mybir.AluOpType
    AX = mybir.AxisListType

    k_iters = cfg["k_iters"]
    mm_dt = F32R if cfg["f32r"] else F32
    c_pad = cfg["c_pad"]          # 0 => dense; else padded gather count
    sparse = c_pad > 0
    n_chunk = (c_pad // 128) if sparse else N_CHUNK
    G_ROWS = 256                  # max rows per dma_gather instruction
    assert not sparse or c_pad % 128 == 0
    if sparse:
        g_sizes = [G_ROWS] * (c_pad // G_ROWS)
        if c_pad % G_ROWS:
            g_sizes.append(c_pad % G_ROWS)
    else:
        g_sizes = []

    nc = bacc.Bacc("TRN2", target_bir_lowering=False, num_devices=N_CORES)

    pk = nc.dram_tensor("proj_key", [B_LOC, S, H], F32, kind="ExternalInput").ap()
    val = nc.dram_tensor("value", [B_LOC, S, KV], mm_dt, kind="ExternalInput").ap()
    wqt = nc.dram_tensor("wq_t", [Q, H], mm_dt, kind="ExternalInput").ap()
    qpt = nc.dram_tensor("query_pt", [128, (Q // 128) * B_LOC], mm_dt,
                         kind="ExternalInput").ap()
    wen = nc.dram_tensor("w_energy", [H], F32, kind="ExternalInput").ap()
    mb = nc.dram_tensor("mask_bias", [B_LOC, 128, n_chunk], F32,
                        kind="ExternalInput").ap()
    if sparse:
        idx = nc.dram_tensor("gather_idx", [B_LOC, 128, c_pad // 16], I16,
                             kind="ExternalInput").ap()
    ctx_o = nc.dram_tensor("ctx", [B_LOC, KV], F32, kind="ExternalOutput").ap()
    al_o = nc.dram_tensor("alphas_c", [B_LOC, 128, n_chunk], F32,
                          kind="ExternalOutput").ap()

    with tile.TileContext(nc) as tc:
        from contextlib import ExitStack
        with ExitStack() as ctx:
            const = ctx.enter_context(tc.tile_pool(name="const", bufs=1))
            dma_bufs = 4 if sparse else 3
            pkp = ctx.enter_context(tc.tile_pool(name="pkp", bufs=dma_bufs))
            vp = ctx.enter_context(tc.tile_pool(name="vp", bufs=dma_bufs))
            sump = ctx.enter_context(tc.tile_pool(name="sump", bufs=3))
            tanhp = ctx.enter_context(tc.tile_pool(name="tanhp", bufs=3))
            smalls = ctx.enter_context(tc.tile_pool(name="smalls", bufs=2))
            outp = ctx.enter_context(tc.tile_pool(name="outp", bufs=2))
            psq = ctx.enter_context(tc.tile_pool(name="psq", bufs=1, space="PSUM"))
            pstot = ctx.enter_context(tc.tile_pool(name="pstot", bufs=1, space="PSUM"))
            psctx = ctx.enter_context(tc.tile_pool(name="psctx", bufs=1, space="PSUM"))

            # ---- constants (loaded once, reused across iterations) ----
            wqt_sb = const.tile([128, Q // 128, H], mm_dt)   # (p, t, h) = wq_t[t*128+p, h]
            nc.sync.dma_start(wqt_sb[:], wqt.rearrange("(t p) h -> p t h", p=128))
            qpt_sb = const.tile([128, (Q // 128) * B_LOC], mm_dt)
            nc.sync.dma_start(qpt_sb[:], qpt[:])
            wb_sb = const.tile([128, H], F32)                # w_energy bcast over partitions
            nc.gpsimd.dma_start(
                out=wb_sb[:],
                in_=bass.AP(tensor=wen.tensor, offset=wen.offset,
                            ap=[[0, 128]] + list(wen.ap)))
            ones_sb = const.tile([128, 128], F32)
            nc.vector.memset(ones_sb[:], 1.0)
            ones1_sb = const.tile([1, 128], F32)
            nc.vector.memset(ones1_sb[:], 1.0)
            mb_sb = const.tile([128, B_LOC, n_chunk], F32)
            for b in range(B_LOC):
                nc.sync.dma_start(mb_sb[:, b, :], mb[b])
            if sparse:
                idx_sb = const.tile([128, B_LOC, c_pad // 16], I16)
                for b in range(B_LOC):
                    nc.sync.dma_start(idx_sb[:, b, :], idx[b])

            for it in range(k_iters):
                for b in range(B_LOC):
                    # ---- q = query[b] @ Wq.T, as [1, H] in PSUM ----
                    ps_q = psq.tile([1, H], F32, tag="psq_shared")
                    for hc in range(H // 512):
                        for t in range(Q // 128):
                            nc.tensor.matmul(
                                ps_q[0:1, hc * 512:(hc + 1) * 512],
                                lhsT=qpt_sb[:, (t * B_LOC + b):(t * B_LOC + b + 1)],
                                rhs=wqt_sb[:, t, hc * 512:(hc + 1) * 512],
                                start=(t == 0), stop=(t == Q // 128 - 1))
                    q_sb = smalls.tile([1, H], F32)
                    nc.vector.tensor_copy(q_sb[:], ps_q[:])
                    # broadcast across partitions: ones[1,128].T @ q_sb[1,H]
                    ps_qb = psq.tile([128, H], F32, tag="psq_shared")
                    for kk in range(H // 512):
                        nc.tensor.matmul(ps_qb[:, kk * 512:(kk + 1) * 512],
                                         lhsT=ones1_sb[:],
                                         rhs=q_sb[:, kk * 512:(kk + 1) * 512],
                                         start=True, stop=True)
                    qb_sb = smalls.tile([128, H], F32)
                    nc.vector.tensor_copy(qb_sb[:], ps_qb[:])

                    exp_sb = smalls.tile([128, n_chunk], mm_dt)
                    scores_sb = smalls.tile([128, n_chunk], F32)
                    ps_ctx = psctx.tile([1, KV], F32)

                    if sparse:
                        # gather unmasked rows, up to G_ROWS per dma_gather
                        pk_g = []
                        v_g = []
                        goff = 0
                        for gs in g_sizes:
                            i0, i1 = goff // 16, (goff + gs) // 16
                            pkt = pkp.tile([128, G_ROWS // 128, H], F32,
                                           tag="pk_gather")
                            nc.gpsimd.dma_gather(
                                out_ap=pkt[:, 0:gs // 128, :], in_ap=pk[b],
                                idxs_ap=idx_sb[:, b, i0:i1],
                                num_idxs=gs, num_idxs_reg=gs,
                                elem_size=H, elem_step=H)
                            vt = vp.tile([128, G_ROWS // 128, KV], mm_dt,
                                         tag="v_gather")
                            nc.gpsimd.dma_gather(
                                out_ap=vt[:, 0:gs // 128, :], in_ap=val[b],
                                idxs_ap=idx_sb[:, b, i0:i1],
                                num_idxs=gs, num_idxs_reg=gs,
                                elem_size=KV, elem_step=KV)
                            for cc in range(gs // 128):
                                pk_g.append(pkt[:, cc, :])
                                v_g.append(vt[:, cc, :])
                            goff += gs

                    for c in range(n_chunk):
                        if sparse:
                            pk_t = pk_g[c]
                            v_t = v_g[c]
                        else:
                            pk_t_full = pkp.tile([128, H], F32)
                            nc.sync.dma_start(pk_t_full[:], pk[b, c * 128:(c + 1) * 128, :])
                            v_t_full = vp.tile([128, KV], mm_dt)
                            nc.scalar.dma_start(v_t_full[:], val[b, c * 128:(c + 1) * 128, :])
                            pk_t = pk_t_full[:]
                            v_t = v_t_full[:]

                        sum_t = sump.tile([128, H], F32)
                        nc.vector.tensor_add(sum_t[:], pk_t, qb_sb[:])
                        tanh_t = tanhp.tile([128, H], F32)
                        nc.scalar.activation(tanh_t[:], sum_t[:], AF.Tanh)
                        # sum_t reused as dummy elementwise output;
                        # accum_out = sum_h(tanh * w_energy) in one DVE pass
                        nc.vector.scalar_tensor_tensor(
                            out=sum_t[:], in0=tanh_t[:], scalar=1.0,
                            in1=wb_sb[:], op0=ALU.mult, op1=ALU.mult,
                            accum_out=scores_sb[:, c:c + 1])
                        # exp(scores + mask_bias); masked/pad rows -> exp(-1e30) = 0
                        nc.scalar.activation(exp_sb[:, c:c + 1], scores_sb[:, c:c + 1],
                                             AF.Exp, bias=mb_sb[:, b, c:c + 1])
                        for k in range(KV // 512):
                            nc.tensor.matmul(
                                ps_ctx[0:1, k * 512:(k + 1) * 512],
                                lhsT=exp_sb[:, c:c + 1],
                                rhs=v_t[:, k * 512:(k + 1) * 512],
                                start=(c == 0), stop=(c == n_chunk - 1))

                    # ---- softmax denominator + normalization ----
                    sums = smalls.tile([128, 1], F32)
                    exp_f32 = exp_sb[:].bitcast(F32)
                    nc.vector.reduce_sum(sums[:], exp_f32, axis=AX.X)
                    ps_tot = pstot.tile([128, 1], F32)
                    nc.tensor.matmul(ps_tot[:], lhsT=ones_sb[:], rhs=sums[:],
                                     start=True, stop=True)
                    inv_sb = smalls.tile([128, 1], F32)
                    nc.vector.reciprocal(inv_sb[:], ps_tot[:])
                    al_sb = outp.tile([128, n_chunk], F32)
                    nc.vector.tensor_scalar_mul(al_sb[:], exp_f32, inv_sb[:, 0:1])
                    nc.sync.dma_start(al_o[b], al_sb[:])
                    for k in range(KV // 512):
                        ctx_sb = outp.tile([1, 512], F32)
                        nc.vector.tensor_scalar_mul(
                            ctx_sb[:], ps_ctx[0:1, k * 512:(k + 1) * 512],
                            inv_sb[0:1, 0:1])
                        nc.sync.dma_start(ctx_o[b:b + 1, k * 512:(k + 1) * 512],
                                          ctx_sb[:])

    nc.compile()
    return nc


def _make_runner(cfg):
    """Compile once; return f(in_maps) -> (fn, dargs-builder) reusable callable."""
    import jax
    from jax.sharding import Mesh, PartitionSpec
    from jax.experimental.shard_map import shard_map
    from concourse import mybir
    from concourse.bass2jax import (_bass_exec_p, partition_id_tensor,
                                    install_neuronx_cc_hook)

    nc = _build(cfg)
    install_neuronx_cc_hook()
    partition_name = nc.partition_id_tensor.name if nc.partition_id_tensor else None
    in_names, out_names, out_avals, zero_outs = [], [], [], []
    for alloc in nc.m.functions[0].allocations:
        if not isinstance(alloc, mybir.MemoryLocationSet):
            continue
        name = alloc.memorylocations[0].name
        if alloc.kind == "ExternalInput":
            if name != partition_name:
                in_names.append(name)
        elif alloc.kind == "ExternalOutput":
            out_names.append(name)
            out_avals.append(jax.core.ShapedArray(
                tuple(alloc.tensor_shape), mybir.dt.np(alloc.dtype)))
            zero_outs.append(np.zeros(tuple(alloc.tensor_shape),
                                      mybir.dt.np(alloc.dtype)))
    n_params = len(in_names)
    all_in = list(in_names) + list(out_names) + (
        [partition_name] if partition_name else [])

    def _body(*args):
        ops = list(args)
        if partition_name:
            ops.append(partition_id_tensor())
        return tuple(_bass_exec_p.bind(
            *ops, out_avals=tuple(out_avals), in_names=tuple(all_in),
            out_names=tuple(out_names), lowering_input_output_aliases=(),
            sim_require_finite=True, sim_require_nnan=True, nc=nc))

    import jax
    devices = jax.devices()[:N_CORES]
    mesh = Mesh(np.asarray(devices), ("core",))
    nio = n_params + len(out_names)
    fn = jax.jit(shard_map(_body, mesh=mesh,
                           in_specs=(PartitionSpec("core"),) * nio,
                           out_specs=(PartitionSpec("core"),) * len(out_names),
                           check_rep=False))

    def _stage(in_maps):
        from jax.sharding import NamedSharding
        sh = NamedSharding(mesh, PartitionSpec("core"))

        def to_global(per_core_arrs):
            """Build a sharded global array from per-device shards without
            going through jax's _multi_slice path."""
            shards = [jax.device_put(a, d)
                      for a, d in zip(per_core_arrs, devices)]
            shape = (sum(a.shape[0] for a in per_core_arrs),) + per_core_arrs[0].shape[1:]
            return jax.make_array_from_single_device_arrays(shape, sh, shards)

        dargs = []
        for nm in in_names:
            dargs.append(to_global([np.asarray(m[nm]) for m in in_maps]))
        for z in zero_outs:
            dargs.append(to_global([z] * N_CORES))
        jax.block_until_ready(dargs)
        return dargs

    def make_call(in_maps):
        dargs = _stage(in_maps)

        def call():
            t0 = time.perf_counter()
            outs = fn(*dargs)
            jax.block_until_ready(outs)
            return time.perf_counter() - t0, outs
        return call

    def run(in_maps, reps=1):
        call = make_call(in_maps)
        call()                               # warm call
        times = []
        for _ in range(reps):
            dt, outs = call()
            times.append(dt)
        res = []
        for c in range(N_CORES):
            m = {}
            for i, nm in enumerate(out_names):
                arr = np.asarray(outs[i])
                per = arr.shape[0] // N_CORES
                m[nm] = arr[c * per:(c + 1) * per]
            res.append(m)
        return res, times

    run.make_call = make_call
    return run


def get_runner(k_iters=1, f32r=True, c_pad=0):
    key = (k_iters, f32r, c_pad)
    if key not in _RUNNER_CACHE:
        _RUNNER_CACHE[key] = _make_runner(
            {"k_iters": k_iters, "f32r": f32r, "c_pad": c_pad})
    return _RUNNER_CACHE[key]


def prep_inputs(mask, query, proj_key, value, Wq, w_energy, c_pad=0):
    """Host-side input prep + per-core sharding. Returns in_maps list."""
    mask = np.asarray(mask)
    query = np.asarray(query, dtype=np.float32)
    proj_key = np.asarray(proj_key, dtype=np.float32)
    value = np.asarray(value, dtype=np.float32)
    wq_t = np.ascontiguousarray(np.asarray(Wq, dtype=np.float32).T)     # [Q, H]
    w_energy = np.ascontiguousarray(np.asarray(w_energy, dtype=np.float32))
    sparse = c_pad > 0
    n_chunk = (c_pad // 128) if sparse else N_CHUNK
    in_maps = []
    for core in range(N_CORES):
        bs = slice(core * B_LOC, (core + 1) * B_LOC)
        qc = query[bs, 0, :].reshape(B_LOC, Q // 128, 128)
        query_pt = np.ascontiguousarray(np.transpose(qc, (2, 1, 0))
                                        ).reshape(128, (Q // 128) * B_LOC)
        m = {
            "proj_key": np.ascontiguousarray(proj_key[bs]),
            "value": np.ascontiguousarray(value[bs]),
            "wq_t": wq_t,
            "query_pt": query_pt,
            "w_energy": w_energy,
        }
        if sparse:
            mbias = np.full((B_LOC, 128, n_chunk), NEG, dtype=np.float32)
            gidx = np.zeros((B_LOC, 128, c_pad // 16), dtype=np.int16)
            for b in range(B_LOC):
                rows = np.nonzero(mask[core * B_LOC + b, 0] != 0)[0]
                cnt = len(rows)
                assert cnt <= c_pad, f"c_pad {c_pad} < count {cnt}"
                padded = np.zeros(c_pad, dtype=np.int16)
                padded[:cnt] = rows
                # pad slots repeat row 0 (always transferred; bias kills them)
                wrapped = padded.reshape(c_pad // 16, 16).T     # [16, c_pad/16]
                # replicated into every 16-partition group (one per Q7 core)
                gidx[b] = np.tile(wrapped, (8, 1))
                mobias = np.full(c_pad, NEG, dtype=np.float32)
                mobias[:cnt] = 0.0
                mbias[b] = mobias.reshape(n_chunk, 128).T       # [128, n_chunk]
            m["mask_bias"] = mbias
            m["gather_idx"] = gidx
        else:
            mm = mask[bs, 0, :].reshape(B_LOC, N_CHUNK, 128)
            mbias = np.where(np.transpose(mm, (0, 2, 1)) == 0, NEG, 0.0
                             ).astype(np.float32)               # [B_LOC,128,n_chunk]
            m["mask_bias"] = np.ascontiguousarray(mbias)
        in_maps.append(m)
    return in_maps


def assemble_outputs(results, mask, c_pad=0):
    """results: per-core dicts with ctx [B_LOC,KV], alphas_c [B_LOC,128,n_chunk]."""
    context = np.zeros((B, 1, KV), dtype=np.float32)
    alphas = np.zeros((B, 1, S), dtype=np.float32)
    sparse = c_pad > 0
    for core in range(N_CORES):
        r = results[core]
        for b in range(B_LOC):
            gb = core * B_LOC + b
            context[gb, 0, :] = r["ctx"][b]
            a = r["alphas_c"][b]                   # [128, n_chunk]
            flat = a.T.reshape(-1)                 # slot i = a[i%128, i//128]
            if sparse:
                rows = np.nonzero(np.asarray(mask)[gb, 0] != 0)[0]
                alphas[gb, 0, rows] = flat[:len(rows)]
            else:
                alphas[gb, 0, :] = flat
    return context, alphas


def pick_c_pad(mask):
    """Smallest multiple of 128 >= max unmasked count over all batches."""
    counts = (np.asarray(mask)[:, 0, :] != 0).sum(axis=1)
    cmax = int(counts.max())
    return ((cmax + 127) // 128) * 128


# Default mode for grading: sparse gather if it fits, else dense.
KERNEL_MODE = os.environ.get("ATT_KERNEL_MODE", "sparse")


def kernel(mask, query, proj_key, value, Wq, w_energy):
    c_pads = []
    if KERNEL_MODE == "sparse":
        c_pads.append(pick_c_pad(mask))
    c_pads.append(0)          # dense fallback
    last_err = None
    for c_pad in c_pads:
        try:
            run = get_runner(k_iters=1, f32r=True, c_pad=c_pad)
            in_maps = prep_inputs(mask, query, proj_key, value, Wq, w_energy,
                                  c_pad=c_pad)
            results, _ = run(in_maps, reps=1)
            return assemble_outputs(results, mask, c_pad=c_pad)
        except Exception as e:          # fall back to dense on any failure
            last_err = e
    raise last_err
